# revision 23
# baseline (speedup 1.0000x reference)
"""MultiHeadAttention Trainium2 kernel, 8-way sharded (batch x head-group).

Sharding: core = 4*b + g  (b in {0,1} batch, g in {0..3} head-group of 4 heads).
Data parallel on batch; tensor parallel on heads for the Q/K/V projections
with a row-parallel Wo output projection.  Each core computes a full-shape
partial output for its batch (bias folded in on one core per group); the
host-side unshard step sums the 4 head-group partials per batch.

Device-side structure (all bf16 matmul operands, f32 PSUM accumulate):
  - x arrives pre-transposed and pre-cast on the host (xT [1024, 2048] bf16
    per batch): zero on-device transposes or input casts.  Weights arrive
    bf16 in SBUF-native layouts (one contiguous DMA line per partition).
  - Startup: a DMA-independent memset tile feeds warm-up matmuls so the PE
    HAM un-throttles while the first weight/x DMAs are still in flight;
    weight loads are split into pieces across the scalar HWDGE queue while
    all x chunks ride the sync HWDGE queue, so the first projection starts
    as soon as its first pieces land.
  - Software pipeline over 512-row chunks, attention ascending 0..3; the
    K/V/Q projections of later chunks and the output projections of earlier
    chunks are interleaved into the attention kt-steps as small "filler"
    units.  Emission-order gates (`ensure_g`) guarantee a chunk's K/V
    projections are emitted before the kt steps that read them.
  - exp batched over both heads of a pair: scores for 2 heads land in one
    [128,2,512] PSUM tile (2 banks), one ACTIVATE computes both.
  - Softmax denominator folded into PV via an augmented ones column in V
    (row 64 of the PV accumulator).  Each pair's normalize is deferred into
    the NEXT pair's kt=1 slot (after that pair's first scores/exp/masks are
    emitted) so the DVE-FIFO reciprocal chain never head-of-line blocks the
    mask multiplies the next pair's PV needs.  The reciprocal reads the
    denominator row in place (partition 64) - no SBUF-SBUF DMA hop.
  - Tail (last chunk, second pair): normalize runs directly out of PSUM,
    split into 128-column blocks so each output-projection block starts as
    soon as its s1T block is written; heater matmuls keep the PE HAM warm
    across the normalize window; final output DMAs alternate queues.
"""
import sys

for _p in ("/opt/trn_rl_repo",):
    if _p not in sys.path:
        sys.path.insert(0, _p)

from collections import deque

import numpy as np
import ml_dtypes

import concourse.bass as bass
import concourse.tile as tile
from concourse import bacc, mybir
from concourse.bass_utils import run_bass_kernel_spmd


def _install_ntff_hook_shim():
    """The agent container's antenv lacks axon_hooks; recreate it so
    run_bass_kernel_spmd(trace=True) can profile via the axon .so."""
    import types, contextlib, ctypes, os

    if "antenv.axon_hooks" in sys.modules:
        return
    mod = types.ModuleType("antenv.axon_hooks")
    _store = {"hook": None}
    mod.set_axon_ntff_profile_hook = lambda h: _store.__setitem__("hook", h)
    mod.get_axon_ntff_profile_hook = lambda: _store["hook"]
    sys.modules["antenv.axon_hooks"] = mod

    so_path = "/opt/axon/libaxon_pjrt.so"
    if not os.path.exists(so_path):
        return
    try:
        lib = ctypes.CDLL(so_path)
        if not hasattr(lib, "axon_start_nrt_profile"):
            return
        lib.axon_start_nrt_profile.argtypes = [
            ctypes.POINTER(ctypes.c_int64), ctypes.c_size_t]
        lib.axon_start_nrt_profile.restype = ctypes.c_int64
        lib.axon_stop_nrt_profile.argtypes = [ctypes.c_char_p]
        lib.axon_stop_nrt_profile.restype = ctypes.c_int64

        @contextlib.contextmanager
        def _hook(output_dir, device_ids):
            import jax
            jax.devices()
            if device_ids:
                ids = (ctypes.c_int64 * len(device_ids))(*device_ids)
                rc = lib.axon_start_nrt_profile(ids, len(device_ids))
            else:
                rc = lib.axon_start_nrt_profile(None, 0)
            if rc != 0:
                raise RuntimeError(f"axon_start_nrt_profile rc={rc}")
            try:
                yield
            finally:
                n = lib.axon_stop_nrt_profile(str(output_dir).encode())
                print(f"ntff profile: {n} file(s) written to {output_dir}")

        mod.set_axon_ntff_profile_hook(_hook)
    except Exception:
        pass


_install_ntff_hook_shim()

F32 = mybir.dt.float32
BF16 = mybir.dt.bfloat16
AF = mybir.ActivationFunctionType
ALU = mybir.AluOpType

B, S, D_EMB = 2, 2048, 1024
H, DH = 16, 64
HG = 4              # heads per core
DM_L = HG * DH      # 256 local mid dim
D_OUT = 1024
NCORES = 8
ET = D_EMB // 128   # 8 emb tiles
QC = 4              # q chunks of 512
SCALE = 1.0 / 8.0   # 1/sqrt(DH)

# augmented V layout: per head slice [v(64), one] -> PV output rows 0..63 = O,
# row 64 = softmax denominator (the ones column sums P over keys).
HOFF = [0, 65, 130, 195]
WV_AUG = 272        # 260 used + pad
WV_USED = 260


def _build():
    nc = bacc.Bacc(None, target_bir_lowering=False, num_devices=NCORES)

    # x inputs arrive chunk-major and SBUF-native: partition p holds, for
    # each 512-query chunk c, its ET rows packed contiguously, so every
    # chunk DMA is one max-length contiguous line per partition.
    xqT = nc.declare_dram_parameter("xqT", [128, QC * ET * 512], BF16, isOutput=False)
    xkT = nc.declare_dram_parameter("xkT", [128, QC * ET * 512], BF16, isOutput=False)
    xvT = nc.declare_dram_parameter("xvT", [128, QC * ET * 512], BF16, isOutput=False)
    wq = nc.declare_dram_parameter("wq", [128, ET * DM_L], BF16, isOutput=False)
    wk = nc.declare_dram_parameter("wk", [128, ET * DM_L], BF16, isOutput=False)
    wv = nc.declare_dram_parameter("wv", [128, ET * WV_AUG], BF16, isOutput=False)
    bq = nc.declare_dram_parameter("bq", [DM_L], F32, isOutput=False)
    bk = nc.declare_dram_parameter("bk", [DM_L], F32, isOutput=False)
    bv = nc.declare_dram_parameter("bv", [WV_AUG], F32, isOutput=False)
    wo = nc.declare_dram_parameter("wo", [128, 2 * D_OUT], BF16, isOutput=False)
    bo = nc.declare_dram_parameter("bo", [D_OUT], F32, isOutput=False)
    mtri = nc.declare_dram_parameter("mtri", [128, 128], BF16, isOutput=False)
    out = nc.declare_dram_parameter("out", [S, D_OUT], BF16, isOutput=True)

    with tile.TileContext(nc) as tc:
        _emit(nc, tc, xqT.ap(), xkT.ap(), xvT.ap(), wq.ap(), wk.ap(), wv.ap(),
              bq.ap(), bk.ap(), bv.ap(), wo.ap(), bo.ap(), mtri.ap(), out.ap())
    nc.compile()
    return nc


def _emit(nc, tc, xqT, xkT, xvT, wq, wk, wv, bq, bk, bv, wo, bo, mtri, out):
    from contextlib import ExitStack

    ctx = ExitStack()
    consts = ctx.enter_context(tc.tile_pool(name="consts", bufs=1))
    wpool = ctx.enter_context(tc.tile_pool(name="wpool", bufs=1))
    persist = ctx.enter_context(tc.tile_pool(name="persist", bufs=1))
    xload = ctx.enter_context(tc.tile_pool(name="xload", bufs=7))
    ptp = ctx.enter_context(tc.tile_pool(name="ptp", bufs=3))
    ocpp = ctx.enter_context(tc.tile_pool(name="ocpp", bufs=2))
    smallp = ctx.enter_context(tc.tile_pool(name="smallp", bufs=4))
    outp = ctx.enter_context(tc.tile_pool(name="outp", bufs=4))
    ps_sc = ctx.enter_context(tc.tile_pool(name="ps_sc", bufs=2, space="PSUM"))
    ps_po = ctx.enter_context(tc.tile_pool(name="ps_po", bufs=1, space="PSUM"))
    ps_pp = ctx.enter_context(tc.tile_pool(name="ps_pp", bufs=2, space="PSUM"))

    # ---- PE warm-up with NO DMA dependency: memset a tile on the (idle)
    # vector queue, then back-to-back matmuls into ONE PSUM tile (same-tile
    # WAW stays in-order on the PE with no semaphore round-trips, so the PE
    # duty cycle is high enough for HAM to un-throttle) while the first
    # weight/x DMAs are still in flight. ----
    zdummy = consts.tile([128, 128], BF16, name="zdummy")
    nc.vector.memset(zdummy[:], 0.0)
    ones1 = consts.tile([1, 128], BF16, name="ones1")
    nc.vector.memset(ones1[:], 1.0)
    # HAM only un-throttles after ~2 full 4096-cycle windows of continuous
    # PE activity (~7us cold): 64 matmuls guarantees it fires mid-warmup,
    # so the projections start at 2.4 GHz.
    warm_ps = ps_pp.tile([128, 512], F32, tag="pp", name="warm")
    for _ in range(64):
        nc.tensor.matmul(
            warm_ps[:, 0:128], lhsT=zdummy[:], rhs=zdummy[:],
            start=True, stop=True,
        )

    # ---- first-wave loads: wk/wq and the first k/q chunk pieces are
    # interleaved across BOTH HWDGE queues (scalar + sync) so the first
    # projections' inputs land as early as possible. ----
    wk_sb = wpool.tile([128, ET, DM_L], BF16, name="wk")
    wk_r = wk.rearrange("p (t d) -> p t d", t=ET)
    wq_sb = wpool.tile([128, ET, DM_L], BF16, name="wq")
    wq_r = wq.rearrange("p (t d) -> p t d", t=ET)
    xk0 = xload.tile([128, ET, 512], BF16, tag="xT", name="xT_k0")
    xq0 = xload.tile([128, ET, 512], BF16, tag="xT", name="xT_q0")
    xk_src = xkT.rearrange("p (c t s) -> p c t s", c=QC, t=ET)
    xq_src = xqT.rearrange("p (c t s) -> p c t s", c=QC, t=ET)
    xv_src = xvT.rearrange("p (c t s) -> p c t s", c=QC, t=ET)
    # scalar queue: wk0, xk0[2:4], wk1, bk, wq0, xq0[2:4], wq1, bq
    # sync queue:  xk0[0:2], xk0[4:6], xk0[6:8], xq0[0:2], xq0[4:6], xq0[6:8]
    nc.scalar.dma_start(wk_sb[:, 0:4, :], wk_r[:, 0:4, :])
    nc.sync.dma_start(xk0[:, 0:2, :], xk_src[:, 0, 0:2, :])
    nc.scalar.dma_start(xk0[:, 2:4, :], xk_src[:, 0, 2:4, :])
    nc.sync.dma_start(xk0[:, 4:6, :], xk_src[:, 0, 4:6, :])
    nc.scalar.dma_start(wk_sb[:, 4:8, :], wk_r[:, 4:8, :])
    nc.sync.dma_start(xk0[:, 6:8, :], xk_src[:, 0, 6:8, :])
    bk_sb = consts.tile([128, 2], F32, name="bk")
    nc.scalar.dma_start(bk_sb[:], bk.rearrange("(c p) -> p c", p=128))
    nc.scalar.dma_start(wq_sb[:, 0:4, :], wq_r[:, 0:4, :])
    nc.sync.dma_start(xq0[:, 0:2, :], xq_src[:, 0, 0:2, :])
    nc.scalar.dma_start(xq0[:, 2:4, :], xq_src[:, 0, 2:4, :])
    nc.sync.dma_start(xq0[:, 4:6, :], xq_src[:, 0, 4:6, :])
    nc.scalar.dma_start(wq_sb[:, 4:8, :], wq_r[:, 4:8, :])
    nc.sync.dma_start(xq0[:, 6:8, :], xq_src[:, 0, 6:8, :])
    bq_sb = consts.tile([128, 2], F32, name="bq")
    nc.scalar.dma_start(bq_sb[:], bq.rearrange("(c p) -> p c", p=128))

    mtri_sb = consts.tile([128, 128], BF16)
    nc.scalar.dma_start(mtri_sb[:], mtri[:])

    # preload the exp table (first ACTIVATE triggers the table DMA)
    dummy_f32 = consts.tile([1, 16], F32)
    nc.vector.memset(dummy_f32[:], 0.0)
    dummy_o = consts.tile([1, 16], F32)
    nc.scalar.activation(out=dummy_o[:], in_=dummy_f32[:], func=AF.Exp, scale=1.0)

    bv_row = consts.tile([1, WV_AUG], F32, name="bv_row")
    nc.scalar.dma_start(bv_row[:], bv[None, :])
    bo_row = consts.tile([1, D_OUT], F32, name="bo_row")
    nc.scalar.dma_start(bo_row[:], bo[None, :])
    # bf16 copy of bo for the tail's fold-bias-into-matmul path
    bo16 = consts.tile([1, D_OUT], BF16, name="bo16")
    nc.vector.tensor_copy(out=bo16[:], in_=bo_row[:])
    wv_sb = wpool.tile([128, ET, WV_AUG], BF16, name="wv")
    wv_r = wv.rearrange("p (t d) -> p t d", t=ET)
    for j in (0, 1):
        nc.scalar.dma_start(wv_sb[:, j * 4:(j + 1) * 4, :], wv_r[:, j * 4:(j + 1) * 4, :])
    wo_sb = wpool.tile([128, 2, D_OUT], BF16, name="wo")
    nc.scalar.dma_start(wo_sb[:], wo.rearrange("p (t d) -> p t d", t=2))

    # gpsimd: broadcasts computed on-engine (no 512KB broadcast DMA)
    bv_bc = consts.tile([128, WV_AUG], F32, name="bv_bc")
    nc.gpsimd.partition_broadcast(bv_bc[:], bv_row[:])
    bo_bc = consts.tile([128, D_OUT], F32)
    nc.gpsimd.partition_broadcast(bo_bc[:], bo_row[:])

    # ---- persistent attention operands ----
    qT = [persist.tile([128, S], BF16, name=f"qT{i}") for i in range(2)]
    kT = [persist.tile([128, S], BF16, name=f"kT{i}") for i in range(2)]
    v_sb = persist.tile([128, 4 * QC, WV_AUG], BF16)
    s1T = persist.tile([128, 2, S], BF16, name="s1T")

    # ---- remaining xT chunk loads, split across both HWDGE queues:
    # k/v ride sync, q rides scalar (after the weights). ----
    xsrc = {"q": xq_src, "k": xk_src, "v": xv_src}
    xtiles = {("k", 0): xk0, ("q", 0): xq0}

    def load_x(key, qc, parts, eng):
        t = xload.tile([128, ET, 512], BF16, tag="xT", name=f"xT_{key}{qc}")
        src = xsrc[key]
        step = ET // parts
        for j in range(0, ET, step):
            eng.dma_start(t[:, j:j + step, :], src[:, qc, j:j + step, :])
        xtiles[(key, qc)] = t

    load_x("v", 0, 2, nc.sync)
    for qc in range(1, QC):
        load_x("k", qc, 1, nc.sync)
        load_x("q", qc, 1, nc.scalar)
        load_x("v", qc, 1, nc.sync)

    # ---- filler machinery: small PE units interleaved into attention.
    # Emission order IS dependency order for the Tile framework, so a unit
    # that writes data consumed by an attention step must be emitted before
    # that step: `ensure_g` force-pops K/V/Q projection units up to the
    # chunk a kt step is about to read. ----
    fillers = deque()
    pending = {}

    def fadd(cost, fn, gate=None):
        fillers.append((cost, fn, gate))
        if gate is not None:
            pending[gate] = pending.get(gate, 0) + 1

    def pop_one():
        cost, fn, gate = fillers.popleft()
        fn()
        if gate is not None:
            pending[gate] -= 1
        return cost

    def pop_fillers(budget):
        while fillers and budget > 0.0:
            budget -= pop_one()

    def ensure_g(key):
        while fillers and pending.get(key, 0):
            pop_one()

    def drain_fillers():
        while fillers:
            pop_one()

    def add_projT_units(qc, c2, w_sb, b_sb, dst):
        """qT/kT projection for mid-half c2 of chunk qc -> dst[c2] columns."""
        st = {}
        kind = "Q" if dst is qT else "K"
        gate = (kind, qc, c2)
        xt = xtiles[(kind.lower(), qc)]

        def u_start(st=st, xt=xt, c2=c2, w_sb=w_sb):
            pp = ps_pp.tile([128, 512], F32, tag="pp")
            st["pp"] = pp
            nc.tensor.matmul(
                pp[:, 0:512],
                lhsT=w_sb[:, 0, c2 * 128:(c2 + 1) * 128],
                rhs=xt[:, 0, :],
                start=True, stop=False,
            )
        fadd(0.25, u_start, gate)

        for e0 in (1, 3, 5):
            def u_mid(st=st, xt=xt, c2=c2, w_sb=w_sb, e0=e0):
                for ei in (e0, e0 + 1):
                    nc.tensor.matmul(
                        st["pp"][:, 0:512],
                        lhsT=w_sb[:, ei, c2 * 128:(c2 + 1) * 128],
                        rhs=xt[:, ei, :],
                        start=False, stop=False,
                    )
            fadd(0.45, u_mid, gate)

        def u_end(st=st, xt=xt, c2=c2, w_sb=w_sb, dst=dst, qc=qc, b_sb=b_sb):
            nc.tensor.matmul(
                st["pp"][:, 0:512],
                lhsT=w_sb[:, ET - 1, c2 * 128:(c2 + 1) * 128],
                rhs=xt[:, ET - 1, :],
                start=False, stop=True,
            )
            nc.vector.tensor_scalar(
                out=dst[c2][:, qc * 512:(qc + 1) * 512],
                in0=st["pp"][:, 0:512],
                scalar1=b_sb[:, c2:c2 + 1],
                scalar2=None,
                op0=ALU.add,
            )
        fadd(0.45, u_end, gate)

    def add_projV_units(qc, r):
        """V projection for 128-row block r of chunk qc -> v_sb natural."""
        st = {}
        si = 4 * qc + r
        xt = xtiles[("v", qc)]

        def u_start(st=st, xt=xt, r=r):
            pp = ps_pp.tile([128, 512], F32, tag="pp")
            st["pp"] = pp
            nc.tensor.matmul(
                pp[:, 0:WV_USED],
                lhsT=xt[:, 0, r * 128:(r + 1) * 128],
                rhs=wv_sb[:, 0, 0:WV_USED],
                start=True, stop=False,
            )
        fadd(0.15, u_start, ("V", qc, r))

        for e0 in (1, 3, 5):
            def u_mid(st=st, xt=xt, r=r, e0=e0):
                for ei in (e0, e0 + 1):
                    nc.tensor.matmul(
                        st["pp"][:, 0:WV_USED],
                        lhsT=xt[:, ei, r * 128:(r + 1) * 128],
                        rhs=wv_sb[:, ei, 0:WV_USED],
                        start=False, stop=False,
                    )
            fadd(0.3, u_mid, ("V", qc, r))

        def u_end(st=st, xt=xt, r=r, si=si):
            nc.tensor.matmul(
                st["pp"][:, 0:WV_USED],
                lhsT=xt[:, ET - 1, r * 128:(r + 1) * 128],
                rhs=wv_sb[:, ET - 1, 0:WV_USED],
                start=False, stop=True,
            )
            nc.vector.tensor_tensor(
                out=v_sb[:, si, 0:WV_USED],
                in0=st["pp"][:, 0:WV_USED],
                in1=bv_bc[:, 0:WV_USED],
                op=ALU.add,
            )
        fadd(0.3, u_end, ("V", qc, r))

    def add_proj_chunk(qc):
        add_projT_units(qc, 0, wk_sb, bk_sb, kT)
        add_projT_units(qc, 0, wq_sb, bq_sb, qT)
        for r in range(4):
            add_projV_units(qc, r)
        add_projT_units(qc, 1, wk_sb, bk_sb, kT)
        add_projT_units(qc, 1, wq_sb, bq_sb, qT)

    def add_outproj_chunk(qc, use_sc=False, tail=False):
        """Output projection for 512-row block qc; partial rows go straight
        to the output parameter (host sums the 4 head-group partials)."""
        for r in range(4):
            si = 4 * qc + r
            st = {}

            def u_alloc(st=st):
                st["ob"] = outp.tile([128, D_OUT], BF16, tag="ob", name="ob")
            fadd(0.0, u_alloc)

            for half in range(2):
                def u_half(st=st, si=si, half=half, r=r):
                    if use_sc and (r + half) % 2 == 0:
                        ppt = ps_sc.tile([128, 2, 512], F32, tag="st", name="pp2")
                        pp = ppt[:, 0, 0:512]
                    else:
                        ppt = ps_pp.tile([128, 512], F32, tag="pp")
                        pp = ppt[:, 0:512]
                    act_evac = tail and half == 1
                    for c2 in range(2):
                        nc.tensor.matmul(
                            pp,
                            lhsT=s1T[:, c2, si * 128:(si + 1) * 128],
                            rhs=wo_sb[:, c2, half * 512:(half + 1) * 512],
                            start=(c2 == 0), stop=(c2 == 1) and not act_evac,
                        )
                    # fold the output bias in here (nonzero only on core
                    # g==0); at the tail, alternate halves evacuate via the
                    # idle ACT engine (bias folded as a K=1 ones matmul) so
                    # the DVE never gates the final output DMAs
                    if act_evac:
                        nc.tensor.matmul(
                            pp,
                            lhsT=ones1[:],
                            rhs=bo16[:, half * 512:(half + 1) * 512],
                            start=False, stop=True,
                        )
                        nc.scalar.copy(
                            out=st["ob"][:, half * 512:(half + 1) * 512], in_=pp,
                        )
                    else:
                        nc.vector.tensor_tensor(
                            out=st["ob"][:, half * 512:(half + 1) * 512],
                            in0=pp,
                            in1=bo_bc[:, half * 512:(half + 1) * 512],
                            op=ALU.add,
                        )
                fadd(0.45, u_half)

            def u_dma(st=st, si=si, r=r):
                if tail:
                    # split across both HWDGE queues so the drain halves
                    nc.sync.dma_start(
                        out[si * 128:(si + 1) * 128, 0:512], st["ob"][:, 0:512]
                    )
                    nc.scalar.dma_start(
                        out[si * 128:(si + 1) * 128, 512:1024], st["ob"][:, 512:1024]
                    )
                else:
                    nc.scalar.dma_start(out[si * 128:(si + 1) * 128, :], st["ob"][:])
            fadd(0.0, u_dma)

    # ---- normalize: O^T rows / den (den in PV-accumulator row 64).
    # Mid-kernel, emitted AT PAIR END (so all readers of the single-buffered
    # po bank exist before the next pair's po allocation): the denominator
    # row hops partitions 64->0 on the ACT engine (cheap, runs parallel to
    # the DVE copy that frees po) and the reciprocal follows immediately.
    # The gpsimd broadcast and the normalize multiplies are deferred into
    # the NEXT pair's kt=1/kt=2 slots so the DVE FIFO never head-of-line
    # blocks the mask multiplies the next pair's PV needs. ----
    def norm_stage1(qc, p, po):
        den0 = smallp.tile([1, 2, 512], F32, tag="den")
        nc.scalar.copy(out=den0[:], in_=po[64:65, :, :])
        ocp = ocpp.tile([65, 2, 512], F32, tag="ocp")
        nc.vector.tensor_copy(out=ocp[:], in_=po[0:65, :, :])
        rec = smallp.tile([1, 2, 512], F32, tag="rec")
        nc.vector.reciprocal_approx_fast(out=rec[:], in_=den0[:])
        return ocp, rec

    def norm_stage2(st):
        recbc = smallp.tile([64, 2, 512], F32, tag="recbc")
        nc.gpsimd.partition_broadcast(recbc[:], st["rec"][:])
        st["recbc"] = recbc

    def norm_stage3(st):
        qc, p, ocp, recbc = st["qc"], st["p"], st["ocp"], st["recbc"]
        # odd head first: normalize at base 0, DMA to partitions 64..127
        tmp = smallp.tile([64, 512], BF16, tag="otmp")
        nc.vector.tensor_tensor(
            out=tmp[:], in0=ocp[0:64, 1, :], in1=recbc[:, 1, :], op=ALU.mult,
        )
        nc.gpsimd.dma_start(
            s1T[64:128, p, qc * 512:(qc + 1) * 512], tmp[:]
        )
        nc.vector.tensor_tensor(
            out=s1T[0:64, p, qc * 512:(qc + 1) * 512],
            in0=ocp[0:64, 0, :], in1=recbc[:, 0, :], op=ALU.mult,
        )
        if st["after"] is not None:
            st["after"]()

    # Tail variant: nothing reuses po, so normalize straight out of PSUM
    # (no copy), the two heads' den->recip->broadcast chains are pipelined
    # with the odd head FIRST (it gates the partition-shift DMA), the
    # normalize is split into 128-col blocks so each outproj block's s1T
    # region completes (and its odd-head DMA, on the idle sync HWDGE queue,
    # flies) as early as possible, and small heater matmuls anchored on the
    # chain's own artifacts keep the PE HAM warm across the window.
    def emit_normalize_tail(qc, p, po):
        den1 = smallp.tile([1, 512], F32, tag="den")
        nc.scalar.copy(out=den1[:], in_=po[64:65, 1, :])
        rec1 = smallp.tile([1, 512], F32, tag="rec")
        nc.vector.reciprocal_approx_fast(out=rec1[:], in_=den1[:])
        den0 = smallp.tile([1, 512], F32, tag="den")
        nc.scalar.copy(out=den0[:], in_=po[64:65, 0, :])
        recbc1 = smallp.tile([64, 512], F32, tag="recbc")
        nc.gpsimd.partition_broadcast(recbc1[:], rec1[:])
        rec0 = smallp.tile([1, 512], F32, tag="rec")
        nc.vector.reciprocal_approx_fast(out=rec0[:], in_=den0[:])
        recbc0 = smallp.tile([64, 512], F32, tag="recbc")
        nc.gpsimd.partition_broadcast(recbc0[:], rec0[:])
        tmp = smallp.tile([64, 512], BF16, tag="otmp")
        for b in range(4):
            cs = slice(b * 128, (b + 1) * 128)
            col = slice(qc * 512 + b * 128, qc * 512 + (b + 1) * 128)
            nc.vector.tensor_tensor(
                out=tmp[:, cs], in0=po[0:64, 1, cs], in1=recbc1[:, cs],
                op=ALU.mult,
            )
            nc.sync.dma_start(s1T[64:128, p, col], tmp[:, cs])
            nc.vector.tensor_tensor(
                out=s1T[0:64, p, col],
                in0=po[0:64, 0, cs], in1=recbc0[:, cs], op=ALU.mult,
            )
            # heater pings anchored on the just-written blocks (the
            # scheduler cannot hoist them out of the normalize window)
            hp = ps_pp.tile([128, 512], F32, tag="pp")
            nc.tensor.matmul(
                hp[:, 0:128], lhsT=zdummy[0:64, :], rhs=tmp[:, cs],
                start=True, stop=True,
            )
            hp2 = ps_pp.tile([128, 512], F32, tag="pp")
            nc.tensor.matmul(
                hp2[:, 0:128], lhsT=zdummy[0:64, :], rhs=s1T[0:64, p, col],
                start=True, stop=True,
            )

    # deferred-normalize state: stage2 (broadcast) fires at the next pair's
    # kt=1, stage3 (multiplies + outproj enqueue) at its kt=2.
    pending_norm = []

    # ---- attention for one 512-query chunk ----
    def attention_chunk(qc, budget=0.6, last=False):
        n_k = 4 * qc + 4
        for p in range(2):
            ensure_g(("Q", qc, p))
            po = ps_po.tile([128, 2, 512], F32, tag="po")
            pend = []
            pt_cur = None
            for kt in range(n_k):
                ensure_g(("K", kt // 4, p))
                diag = kt >= 4 * qc
                q0 = 128 * (kt - 4 * qc) if diag else 0
                e = kt % 2
                if e == 0:
                    pt_cur = ptp.tile([128, 2, 2, 512], BF16, tag="pt")
                # scores for both heads of the pair: concurrent 64-row groups
                ps = ps_sc.tile([128, 2, 512], F32, tag="st")
                for h in range(2):
                    base = 64 * h
                    nc.tensor.matmul(
                        ps[:, h, q0:512],
                        lhsT=kT[p][base:base + 64, kt * 128:(kt + 1) * 128],
                        rhs=qT[p][base:base + 64, qc * 512 + q0:(qc + 1) * 512],
                        start=True, stop=True,
                    )
                # one exp for both heads
                nc.scalar.activation(
                    out=pt_cur[:, e, :, q0:512], in_=ps[:, :, q0:512],
                    func=AF.Exp, scale=SCALE,
                )
                if diag:
                    for h in range(2):
                        nc.vector.tensor_tensor(
                            out=pt_cur[:, e, h, q0:q0 + 128],
                            in0=pt_cur[:, e, h, q0:q0 + 128],
                            in1=mtri_sb[:],
                            op=ALU.mult,
                        )
                pend.append((kt, pt_cur, e, q0))
                # the previous pair's deferred normalize stages slot in here
                if kt == 1 and pending_norm:
                    norm_stage2(pending_norm[0])
                if kt == 2 and pending_norm:
                    norm_stage3(pending_norm.pop(0))
                # PV one step behind so exp(kt) overlaps scores(kt+1)
                if kt >= 1:
                    pkt, ptt, pe, pq0 = pend.pop(0)
                    ensure_g(("V", pkt // 4, pkt % 4))
                    for h in range(2):
                        hh = 2 * p + h
                        nc.tensor.matmul(
                            po[0:65, h, pq0:512],
                            lhsT=v_sb[:, pkt, HOFF[hh]:HOFF[hh] + 65],
                            rhs=ptt[:, pe, h, pq0:512],
                            start=(pkt == 0), stop=False,
                        )
                pop_fillers(budget)
            # final PV
            pkt, ptt, pe, pq0 = pend.pop(0)
            ensure_g(("V", pkt // 4, pkt % 4))
            for h in range(2):
                hh = 2 * p + h
                nc.tensor.matmul(
                    po[0:65, h, pq0:512],
                    lhsT=v_sb[:, pkt, HOFF[hh]:HOFF[hh] + 65],
                    rhs=ptt[:, pe, h, pq0:512],
                    start=(pkt == 0), stop=True,
                )
            if last and p == 1:
                # heater matmuls anchored on the last pt tile keep HAM warm
                # from the final PV into the normalize chain
                for _ in range(12):
                    hp = ps_pp.tile([128, 512], F32, tag="pp")
                    nc.tensor.matmul(
                        hp[:, 0:512], lhsT=zdummy[:], rhs=ptt[:, pe, 0, 0:512],
                        start=True, stop=True,
                    )
                emit_normalize_tail(qc, p, po)
            else:
                ocp, rec = norm_stage1(qc, p, po)
                pending_norm.append(
                    {"qc": qc, "p": p, "ocp": ocp, "rec": rec, "after": None}
                )

    # ---- the pipeline: ascending chunks; all projections ride along as
    # gated fillers, pulled just ahead of the attention steps that consume
    # them; output projections of earlier chunks fill later windows.  Each
    # chunk's outproj fillers are enqueued right after its second pair's
    # deferred normalize is emitted (emission order = dependency order). ----
    for qc in range(QC):
        add_proj_chunk(qc)
    attention_chunk(0, budget=0.7)
    pending_norm[-1]["after"] = lambda: add_outproj_chunk(0)
    attention_chunk(1, budget=0.7)
    pending_norm[-1]["after"] = lambda: add_outproj_chunk(1)
    attention_chunk(2, budget=0.8)
    pending_norm[-1]["after"] = lambda: add_outproj_chunk(2)
    attention_chunk(3, budget=1.2, last=True)
    drain_fillers()
    add_outproj_chunk(3, use_sc=True, tail=True)
    drain_fillers()

    ctx.close()


_NC_CACHE = None


def _get_nc():
    global _NC_CACHE
    if _NC_CACHE is None:
        _NC_CACHE = _build()
    return _NC_CACHE


def _make_in_maps(x_q, x_k, x_v, Wq, bq, Wk, bk, Wv, bv, Wo, bo):
    f32 = np.float32
    bf16 = ml_dtypes.bfloat16
    mtri_np = np.triu(np.ones((128, 128), f32)).astype(bf16)

    # per-batch transposed inputs (shared by the 4 cores of each batch),
    # chunk-major SBUF-native: [128, QC*ET*512] with partition p holding,
    # per chunk, its ET 512-col rows packed contiguously.
    def x_layout(x):
        a = np.asarray(x, f32).reshape(QC, 512, ET, 128).transpose(3, 0, 2, 1)
        return np.ascontiguousarray(a.reshape(128, QC * ET * 512)).astype(bf16)

    xT = {}
    for b in range(B):
        xT[("q", b)] = x_layout(x_q[b])
        xT[("k", b)] = x_layout(x_k[b])
        xT[("v", b)] = x_layout(x_v[b])

    def sb_layout(w):
        """[D_EMB, n] -> [128, ET*n]: partition p holds rows {t*128+p} packed
        contiguously, so the DMA is one max-length line per partition."""
        n = w.shape[1]
        return np.ascontiguousarray(
            w.reshape(ET, 128, n).transpose(1, 0, 2).reshape(128, ET * n)
        )

    in_maps = []
    for core in range(NCORES):
        b, g = core // 4, core % 4
        sl = slice(g * DM_L, (g + 1) * DM_L)
        # augmented V weight/bias
        wv_aug = np.zeros((D_EMB, WV_AUG), f32)
        bv_aug = np.zeros((WV_AUG,), f32)
        for h in range(HG):
            gh = g * HG + h
            o = HOFF[h]
            wv_aug[:, o:o + 64] = Wv[:, gh * DH:(gh + 1) * DH]
            bv_aug[o:o + 64] = bv[gh * DH:(gh + 1) * DH]
            bv_aug[o + 64] = 1.0
        wo_sl = np.asarray(Wo[sl, :], f32)  # [256, 1024]
        wo_c = np.ascontiguousarray(
            wo_sl.reshape(2, 128, D_OUT).transpose(1, 0, 2).reshape(128, 2 * D_OUT)
        )
        in_maps.append({
            "xqT": xT[("q", b)],
            "xkT": xT[("k", b)],
            "xvT": xT[("v", b)],
            "wq": sb_layout(np.asarray(Wq[:, sl], f32)).astype(bf16),
            "wk": sb_layout(np.asarray(Wk[:, sl], f32)).astype(bf16),
            "wv": sb_layout(wv_aug).astype(bf16),
            "bq": np.ascontiguousarray(bq[sl], f32),
            "bk": np.ascontiguousarray(bk[sl], f32),
            "bv": bv_aug,
            "wo": wo_c.astype(bf16),
            # bias folded into the partials by exactly one core per group
            "bo": np.ascontiguousarray(bo, f32) if g == 0
                  else np.zeros((D_OUT,), f32),
            "mtri": mtri_np,
        })
    return in_maps


def run(inputs, trace=False, trace_kwargs=None):
    """Run on 8 NeuronCores. Returns (output [2,2048,1024] f32, BassKernelResults)."""
    inputs = {k: np.asarray(v) for k, v in inputs.items()}
    nc = _get_nc()
    in_maps = _make_in_maps(
        inputs["x_q"], inputs["x_k"], inputs["x_v"],
        inputs["Wq"], inputs["bq"], inputs["Wk"], inputs["bk"],
        inputs["Wv"], inputs["bv"], inputs["Wo"], inputs["bo"],
    )
    kwargs = {}
    if trace:
        kwargs["trace"] = True
        if trace_kwargs:
            kwargs.update(trace_kwargs)
    res = run_bass_kernel_spmd(nc, in_maps, core_ids=list(range(NCORES)), **kwargs)
    # unshard: each core holds a full-shape row-parallel partial for its
    # batch (4 head-groups per batch); summing them is the unshard step.
    out_full = np.zeros((B, S, D_OUT), np.float32)
    for core in range(NCORES):
        b = core // 4
        out_full[b] += np.asarray(res.results[core]["out"], np.float32)
    return out_full, res


def kernel(**inputs) -> np.ndarray:
    out, _ = run(inputs, trace=False)
    return out


# revision 25
# speedup vs baseline: 1.2077x; 1.2077x over previous
"""MultiHeadAttention Trainium2 kernel, 8-way sharded (batch x head-group).

Sharding: core = 4*b + g  (b in {0,1} batch, g in {0..3} head-group of 4 heads).
Data parallel on batch; tensor parallel on heads for the Q/K/V projections
with a row-parallel Wo output projection.  Each core computes a full-shape
partial output for its batch (bias folded in on one core per group); the
host-side unshard step sums the 4 head-group partials per batch.

Device-side structure (all bf16 matmul operands, f32 PSUM accumulate):
  - x arrives pre-transposed and pre-cast on the host (xT [1024, 2048] bf16
    per batch): zero on-device transposes or input casts.  Weights arrive
    bf16 in SBUF-native layouts (one contiguous DMA line per partition).
  - Startup: a DMA-independent memset tile feeds warm-up matmuls so the PE
    HAM un-throttles while the first weight/x DMAs are still in flight;
    weight loads are split into pieces across the scalar HWDGE queue while
    all x chunks ride the sync HWDGE queue, so the first projection starts
    as soon as its first pieces land.
  - Software pipeline over 512-row chunks, attention ascending 0..3; the
    K/V/Q projections of later chunks and the output projections of earlier
    chunks are interleaved into the attention kt-steps as small "filler"
    units.  Emission-order gates (`ensure_g`) guarantee a chunk's K/V
    projections are emitted before the kt steps that read them.
  - exp batched over both heads of a pair: scores for 2 heads land in one
    [128,2,512] PSUM tile (2 banks), one ACTIVATE computes both.
  - Softmax denominator folded into PV via an augmented ones column in V
    (row 64 of the PV accumulator).  Each pair's normalize is deferred into
    the NEXT pair's kt=1 slot (after that pair's first scores/exp/masks are
    emitted) so the DVE-FIFO reciprocal chain never head-of-line blocks the
    mask multiplies the next pair's PV needs.  The reciprocal reads the
    denominator row in place (partition 64) - no SBUF-SBUF DMA hop.
  - Tail (last chunk, second pair): normalize runs directly out of PSUM,
    split into 128-column blocks so each output-projection block starts as
    soon as its s1T block is written; heater matmuls keep the PE HAM warm
    across the normalize window; final output DMAs alternate queues.
"""
import sys

for _p in ("/opt/trn_rl_repo",):
    if _p not in sys.path:
        sys.path.insert(0, _p)

from collections import deque

import numpy as np
import ml_dtypes

import concourse.bass as bass
import concourse.tile as tile
from concourse import bacc, mybir
from concourse.bass_utils import run_bass_kernel_spmd


def _install_ntff_hook_shim():
    """The agent container's antenv lacks axon_hooks; recreate it so
    run_bass_kernel_spmd(trace=True) can profile via the axon .so."""
    import types, contextlib, ctypes, os

    if "antenv.axon_hooks" in sys.modules:
        return
    mod = types.ModuleType("antenv.axon_hooks")
    _store = {"hook": None}
    mod.set_axon_ntff_profile_hook = lambda h: _store.__setitem__("hook", h)
    mod.get_axon_ntff_profile_hook = lambda: _store["hook"]
    sys.modules["antenv.axon_hooks"] = mod

    so_path = "/opt/axon/libaxon_pjrt.so"
    if not os.path.exists(so_path):
        return
    try:
        lib = ctypes.CDLL(so_path)
        if not hasattr(lib, "axon_start_nrt_profile"):
            return
        lib.axon_start_nrt_profile.argtypes = [
            ctypes.POINTER(ctypes.c_int64), ctypes.c_size_t]
        lib.axon_start_nrt_profile.restype = ctypes.c_int64
        lib.axon_stop_nrt_profile.argtypes = [ctypes.c_char_p]
        lib.axon_stop_nrt_profile.restype = ctypes.c_int64

        @contextlib.contextmanager
        def _hook(output_dir, device_ids):
            import jax
            jax.devices()
            if device_ids:
                ids = (ctypes.c_int64 * len(device_ids))(*device_ids)
                rc = lib.axon_start_nrt_profile(ids, len(device_ids))
            else:
                rc = lib.axon_start_nrt_profile(None, 0)
            if rc != 0:
                raise RuntimeError(f"axon_start_nrt_profile rc={rc}")
            try:
                yield
            finally:
                n = lib.axon_stop_nrt_profile(str(output_dir).encode())
                print(f"ntff profile: {n} file(s) written to {output_dir}")

        mod.set_axon_ntff_profile_hook(_hook)
    except Exception:
        pass


_install_ntff_hook_shim()

F32 = mybir.dt.float32
BF16 = mybir.dt.bfloat16
AF = mybir.ActivationFunctionType
ALU = mybir.AluOpType

B, S, D_EMB = 2, 2048, 1024
H, DH = 16, 64
HG = 4              # heads per core
DM_L = HG * DH      # 256 local mid dim
D_OUT = 1024
NCORES = 8
ET = D_EMB // 128   # 8 emb tiles
QC = 4              # q chunks of 512
SCALE = 1.0 / 8.0   # 1/sqrt(DH)

# augmented V layout: per head slice [v(64), one] -> PV output rows 0..63 = O,
# row 64 = softmax denominator (the ones column sums P over keys).
HOFF = [0, 65, 130, 195]
WV_AUG = 272        # 260 used + pad
WV_USED = 260


def _build():
    nc = bacc.Bacc(None, target_bir_lowering=False, num_devices=NCORES)

    # x inputs arrive chunk-major and SBUF-native: partition p holds, for
    # each 512-query chunk c, its ET rows packed contiguously, so every
    # chunk DMA is one max-length contiguous line per partition.
    xqT = nc.declare_dram_parameter("xqT", [128, QC * ET * 512], BF16, isOutput=False)
    xkT = nc.declare_dram_parameter("xkT", [128, QC * ET * 512], BF16, isOutput=False)
    xvT = nc.declare_dram_parameter("xvT", [128, QC * ET * 512], BF16, isOutput=False)
    wq = nc.declare_dram_parameter("wq", [128, ET * DM_L], BF16, isOutput=False)
    wk = nc.declare_dram_parameter("wk", [128, ET * DM_L], BF16, isOutput=False)
    wv = nc.declare_dram_parameter("wv", [128, ET * WV_AUG], BF16, isOutput=False)
    bq = nc.declare_dram_parameter("bq", [DM_L], F32, isOutput=False)
    bk = nc.declare_dram_parameter("bk", [DM_L], F32, isOutput=False)
    bv = nc.declare_dram_parameter("bv", [WV_AUG], F32, isOutput=False)
    wo = nc.declare_dram_parameter("wo", [128, 2 * D_OUT], BF16, isOutput=False)
    bo = nc.declare_dram_parameter("bo", [D_OUT], F32, isOutput=False)
    mtri = nc.declare_dram_parameter("mtri", [128, 128], BF16, isOutput=False)
    out = nc.declare_dram_parameter("out", [S, D_OUT], BF16, isOutput=True)

    with tile.TileContext(nc) as tc:
        _emit(nc, tc, xqT.ap(), xkT.ap(), xvT.ap(), wq.ap(), wk.ap(), wv.ap(),
              bq.ap(), bk.ap(), bv.ap(), wo.ap(), bo.ap(), mtri.ap(), out.ap())
    nc.compile()
    return nc


def _emit(nc, tc, xqT, xkT, xvT, wq, wk, wv, bq, bk, bv, wo, bo, mtri, out):
    from contextlib import ExitStack

    ctx = ExitStack()
    consts = ctx.enter_context(tc.tile_pool(name="consts", bufs=1))
    wpool = ctx.enter_context(tc.tile_pool(name="wpool", bufs=1))
    persist = ctx.enter_context(tc.tile_pool(name="persist", bufs=1))
    xload = ctx.enter_context(tc.tile_pool(name="xload", bufs=7))
    ptp = ctx.enter_context(tc.tile_pool(name="ptp", bufs=3))
    ocpp = ctx.enter_context(tc.tile_pool(name="ocpp", bufs=2))
    smallp = ctx.enter_context(tc.tile_pool(name="smallp", bufs=4))
    outp = ctx.enter_context(tc.tile_pool(name="outp", bufs=4))
    ps_sc = ctx.enter_context(tc.tile_pool(name="ps_sc", bufs=2, space="PSUM"))
    ps_po = ctx.enter_context(tc.tile_pool(name="ps_po", bufs=1, space="PSUM"))
    ps_pp = ctx.enter_context(tc.tile_pool(name="ps_pp", bufs=2, space="PSUM"))

    # ---- PE warm-up with NO DMA dependency: memset a tile on the (idle)
    # vector queue, then back-to-back matmuls into ONE PSUM tile (same-tile
    # WAW stays in-order on the PE with no semaphore round-trips, so the PE
    # duty cycle is high enough for HAM to un-throttle) while the first
    # weight/x DMAs are still in flight. ----
    zdummy = consts.tile([128, 128], BF16, name="zdummy")
    nc.vector.memset(zdummy[:], 0.0)
    ones1 = consts.tile([1, 128], BF16, name="ones1")
    nc.vector.memset(ones1[:], 1.0)
    # HAM only un-throttles after ~2 full 4096-cycle windows of continuous
    # PE activity (~7us cold): 64 matmuls guarantees it fires mid-warmup,
    # so the projections start at 2.4 GHz.
    warm_ps = ps_pp.tile([128, 512], F32, tag="pp", name="warm")
    for _ in range(64):
        nc.tensor.matmul(
            warm_ps[:, 0:128], lhsT=zdummy[:], rhs=zdummy[:],
            start=True, stop=True,
        )

    # ---- first-wave loads: wk/wq and the first k/q chunk pieces are
    # interleaved across BOTH HWDGE queues (scalar + sync) so the first
    # projections' inputs land as early as possible. ----
    wk_sb = wpool.tile([128, ET, DM_L], BF16, name="wk")
    wk_r = wk.rearrange("p (t d) -> p t d", t=ET)
    wq_sb = wpool.tile([128, ET, DM_L], BF16, name="wq")
    wq_r = wq.rearrange("p (t d) -> p t d", t=ET)
    xk0 = xload.tile([128, ET, 512], BF16, tag="xT", name="xT_k0")
    xq0 = xload.tile([128, ET, 512], BF16, tag="xT", name="xT_q0")
    xk_src = xkT.rearrange("p (c t s) -> p c t s", c=QC, t=ET)
    xq_src = xqT.rearrange("p (c t s) -> p c t s", c=QC, t=ET)
    xv_src = xvT.rearrange("p (c t s) -> p c t s", c=QC, t=ET)
    # scalar queue: wk0, xk0[2:4], wk1, bk, wq0, xq0[2:4], wq1, bq
    # sync queue:  xk0[0:2], xk0[4:6], xk0[6:8], xq0[0:2], xq0[4:6], xq0[6:8]
    nc.scalar.dma_start(wk_sb[:, 0:4, :], wk_r[:, 0:4, :])
    nc.sync.dma_start(xk0[:, 0:2, :], xk_src[:, 0, 0:2, :])
    nc.scalar.dma_start(xk0[:, 2:4, :], xk_src[:, 0, 2:4, :])
    nc.sync.dma_start(xk0[:, 4:6, :], xk_src[:, 0, 4:6, :])
    nc.scalar.dma_start(wk_sb[:, 4:8, :], wk_r[:, 4:8, :])
    nc.sync.dma_start(xk0[:, 6:8, :], xk_src[:, 0, 6:8, :])
    bk_sb = consts.tile([128, 2], F32, name="bk")
    nc.scalar.dma_start(bk_sb[:], bk.rearrange("(c p) -> p c", p=128))
    nc.scalar.dma_start(wq_sb[:, 0:4, :], wq_r[:, 0:4, :])
    nc.sync.dma_start(xq0[:, 0:2, :], xq_src[:, 0, 0:2, :])
    nc.scalar.dma_start(xq0[:, 2:4, :], xq_src[:, 0, 2:4, :])
    nc.sync.dma_start(xq0[:, 4:6, :], xq_src[:, 0, 4:6, :])
    nc.scalar.dma_start(wq_sb[:, 4:8, :], wq_r[:, 4:8, :])
    nc.sync.dma_start(xq0[:, 6:8, :], xq_src[:, 0, 6:8, :])
    bq_sb = consts.tile([128, 2], F32, name="bq")
    nc.scalar.dma_start(bq_sb[:], bq.rearrange("(c p) -> p c", p=128))

    mtri_sb = consts.tile([128, 128], BF16)
    nc.scalar.dma_start(mtri_sb[:], mtri[:])

    # preload the exp table (first ACTIVATE triggers the table DMA)
    dummy_f32 = consts.tile([1, 16], F32)
    nc.vector.memset(dummy_f32[:], 0.0)
    dummy_o = consts.tile([1, 16], F32)
    nc.scalar.activation(out=dummy_o[:], in_=dummy_f32[:], func=AF.Exp, scale=1.0)

    bv_row = consts.tile([1, WV_AUG], F32, name="bv_row")
    nc.scalar.dma_start(bv_row[:], bv[None, :])
    bo_row = consts.tile([1, D_OUT], F32, name="bo_row")
    nc.scalar.dma_start(bo_row[:], bo[None, :])
    # bf16 copy of bo for the tail's fold-bias-into-matmul path; the DVE
    # copy is emitted late (see pipeline) so it never head-of-line blocks
    # the projection evacuations behind bo_row's DMA.
    bo16 = consts.tile([1, D_OUT], BF16, name="bo16")
    wv_sb = wpool.tile([128, ET, WV_AUG], BF16, name="wv")
    wv_r = wv.rearrange("p (t d) -> p t d", t=ET)
    for j in (0, 1):
        nc.scalar.dma_start(wv_sb[:, j * 4:(j + 1) * 4, :], wv_r[:, j * 4:(j + 1) * 4, :])
    wo_sb = wpool.tile([128, 2, D_OUT], BF16, name="wo")
    nc.scalar.dma_start(wo_sb[:], wo.rearrange("p (t d) -> p t d", t=2))

    # gpsimd: broadcasts computed on-engine (no 512KB broadcast DMA)
    bv_bc = consts.tile([128, WV_AUG], F32, name="bv_bc")
    nc.gpsimd.partition_broadcast(bv_bc[:], bv_row[:])
    bo_bc = consts.tile([128, D_OUT], F32)
    nc.gpsimd.partition_broadcast(bo_bc[:], bo_row[:])

    # ---- persistent attention operands ----
    qT = [persist.tile([128, S], BF16, name=f"qT{i}") for i in range(2)]
    kT = [persist.tile([128, S], BF16, name=f"kT{i}") for i in range(2)]
    v_sb = persist.tile([128, 4 * QC, WV_AUG], BF16)
    s1T = persist.tile([128, 2, S], BF16, name="s1T")

    # ---- remaining xT chunk loads, split across both HWDGE queues:
    # k/v ride sync, q rides scalar (after the weights). ----
    xsrc = {"q": xq_src, "k": xk_src, "v": xv_src}
    xtiles = {("k", 0): xk0, ("q", 0): xq0}

    def load_x(key, qc, parts, eng):
        t = xload.tile([128, ET, 512], BF16, tag="xT", name=f"xT_{key}{qc}")
        src = xsrc[key]
        step = ET // parts
        for j in range(0, ET, step):
            eng.dma_start(t[:, j:j + step, :], src[:, qc, j:j + step, :])
        xtiles[(key, qc)] = t

    load_x("v", 0, 2, nc.sync)
    for qc in range(1, QC):
        load_x("k", qc, 1, nc.sync)
        load_x("q", qc, 1, nc.scalar)
        load_x("v", qc, 1, nc.sync)

    # ---- filler machinery: small PE units interleaved into attention.
    # Emission order IS dependency order for the Tile framework, so a unit
    # that writes data consumed by an attention step must be emitted before
    # that step: `ensure_g` force-pops K/V/Q projection units up to the
    # chunk a kt step is about to read. ----
    fillers = deque()
    pending = {}

    def fadd(cost, fn, gate=None):
        fillers.append((cost, fn, gate))
        if gate is not None:
            pending[gate] = pending.get(gate, 0) + 1

    def pop_one():
        cost, fn, gate = fillers.popleft()
        fn()
        if gate is not None:
            pending[gate] -= 1
        return cost

    def pop_fillers(budget):
        while fillers and budget > 0.0:
            budget -= pop_one()

    def ensure_g(key):
        while fillers and pending.get(key, 0):
            pop_one()

    def drain_fillers():
        while fillers:
            pop_one()

    def add_projT_units(qc, c2, w_sb, b_sb, dst):
        """qT/kT projection for mid-half c2 of chunk qc -> dst[c2] columns."""
        st = {}
        kind = "Q" if dst is qT else "K"
        gate = (kind, qc, c2)
        xt = xtiles[(kind.lower(), qc)]

        def u_start(st=st, xt=xt, c2=c2, w_sb=w_sb):
            pp = ps_pp.tile([128, 512], F32, tag="pp")
            st["pp"] = pp
            nc.tensor.matmul(
                pp[:, 0:512],
                lhsT=w_sb[:, 0, c2 * 128:(c2 + 1) * 128],
                rhs=xt[:, 0, :],
                start=True, stop=False,
            )
        fadd(0.25, u_start, gate)

        for e0 in (1, 3, 5):
            def u_mid(st=st, xt=xt, c2=c2, w_sb=w_sb, e0=e0):
                for ei in (e0, e0 + 1):
                    nc.tensor.matmul(
                        st["pp"][:, 0:512],
                        lhsT=w_sb[:, ei, c2 * 128:(c2 + 1) * 128],
                        rhs=xt[:, ei, :],
                        start=False, stop=False,
                    )
            fadd(0.45, u_mid, gate)

        def u_end(st=st, xt=xt, c2=c2, w_sb=w_sb, dst=dst, qc=qc, b_sb=b_sb):
            nc.tensor.matmul(
                st["pp"][:, 0:512],
                lhsT=w_sb[:, ET - 1, c2 * 128:(c2 + 1) * 128],
                rhs=xt[:, ET - 1, :],
                start=False, stop=True,
            )
            nc.vector.tensor_scalar(
                out=dst[c2][:, qc * 512:(qc + 1) * 512],
                in0=st["pp"][:, 0:512],
                scalar1=b_sb[:, c2:c2 + 1],
                scalar2=None,
                op0=ALU.add,
            )
        fadd(0.45, u_end, gate)

    def add_projV_units(qc, r):
        """V projection for 128-row block r of chunk qc -> v_sb natural."""
        st = {}
        si = 4 * qc + r
        xt = xtiles[("v", qc)]

        def u_start(st=st, xt=xt, r=r):
            pp = ps_pp.tile([128, 512], F32, tag="pp")
            st["pp"] = pp
            nc.tensor.matmul(
                pp[:, 0:WV_USED],
                lhsT=xt[:, 0, r * 128:(r + 1) * 128],
                rhs=wv_sb[:, 0, 0:WV_USED],
                start=True, stop=False,
            )
        fadd(0.15, u_start, ("V", qc, r))

        for e0 in (1, 3, 5):
            def u_mid(st=st, xt=xt, r=r, e0=e0):
                for ei in (e0, e0 + 1):
                    nc.tensor.matmul(
                        st["pp"][:, 0:WV_USED],
                        lhsT=xt[:, ei, r * 128:(r + 1) * 128],
                        rhs=wv_sb[:, ei, 0:WV_USED],
                        start=False, stop=False,
                    )
            fadd(0.3, u_mid, ("V", qc, r))

        def u_end(st=st, xt=xt, r=r, si=si):
            nc.tensor.matmul(
                st["pp"][:, 0:WV_USED],
                lhsT=xt[:, ET - 1, r * 128:(r + 1) * 128],
                rhs=wv_sb[:, ET - 1, 0:WV_USED],
                start=False, stop=True,
            )
            nc.vector.tensor_tensor(
                out=v_sb[:, si, 0:WV_USED],
                in0=st["pp"][:, 0:WV_USED],
                in1=bv_bc[:, 0:WV_USED],
                op=ALU.add,
            )
        fadd(0.3, u_end, ("V", qc, r))

    def add_proj_chunk(qc):
        add_projT_units(qc, 0, wk_sb, bk_sb, kT)
        add_projT_units(qc, 0, wq_sb, bq_sb, qT)
        for r in range(4):
            add_projV_units(qc, r)
        add_projT_units(qc, 1, wk_sb, bk_sb, kT)
        add_projT_units(qc, 1, wq_sb, bq_sb, qT)

    def add_outproj_chunk(qc, use_sc=False, tail=False):
        """Output projection for 512-row block qc; partial rows go straight
        to the output parameter (host sums the 4 head-group partials)."""
        for r in range(4):
            si = 4 * qc + r
            st = {}

            def u_alloc(st=st):
                st["ob"] = outp.tile([128, D_OUT], BF16, tag="ob", name="ob")
            fadd(0.0, u_alloc)

            for half in range(2):
                def u_half(st=st, si=si, half=half, r=r):
                    if use_sc and (r + half) % 2 == 0:
                        ppt = ps_sc.tile([128, 2, 512], F32, tag="st", name="pp2")
                        pp = ppt[:, 0, 0:512]
                    else:
                        ppt = ps_pp.tile([128, 512], F32, tag="pp")
                        pp = ppt[:, 0:512]
                    act_evac = tail and half == 1
                    for c2 in range(2):
                        nc.tensor.matmul(
                            pp,
                            lhsT=s1T[:, c2, si * 128:(si + 1) * 128],
                            rhs=wo_sb[:, c2, half * 512:(half + 1) * 512],
                            start=(c2 == 0), stop=(c2 == 1) and not act_evac,
                        )
                    # fold the output bias in here (nonzero only on core
                    # g==0); at the tail, alternate halves evacuate via the
                    # idle ACT engine (bias folded as a K=1 ones matmul) so
                    # the DVE never gates the final output DMAs
                    if act_evac:
                        nc.tensor.matmul(
                            pp,
                            lhsT=ones1[:],
                            rhs=bo16[:, half * 512:(half + 1) * 512],
                            start=False, stop=True,
                        )
                        nc.scalar.copy(
                            out=st["ob"][:, half * 512:(half + 1) * 512], in_=pp,
                        )
                    else:
                        nc.vector.tensor_tensor(
                            out=st["ob"][:, half * 512:(half + 1) * 512],
                            in0=pp,
                            in1=bo_bc[:, half * 512:(half + 1) * 512],
                            op=ALU.add,
                        )
                fadd(0.45, u_half)

            def u_dma(st=st, si=si, r=r):
                if tail:
                    # split across both HWDGE queues so the drain halves
                    nc.sync.dma_start(
                        out[si * 128:(si + 1) * 128, 0:512], st["ob"][:, 0:512]
                    )
                    nc.scalar.dma_start(
                        out[si * 128:(si + 1) * 128, 512:1024], st["ob"][:, 512:1024]
                    )
                else:
                    nc.scalar.dma_start(out[si * 128:(si + 1) * 128, :], st["ob"][:])
            fadd(0.0, u_dma)

    # ---- normalize: O^T rows / den (den in PV-accumulator row 64).
    # Mid-kernel, emitted AT PAIR END (so all readers of the single-buffered
    # po bank exist before the next pair's po allocation): the denominator
    # row hops partitions 64->0 on the ACT engine (cheap, runs parallel to
    # the DVE copy that frees po) and the reciprocal follows immediately.
    # The gpsimd broadcast and the normalize multiplies are deferred into
    # the NEXT pair's kt=1/kt=2 slots so the DVE FIFO never head-of-line
    # blocks the mask multiplies the next pair's PV needs. ----
    def norm_stage1(qc, p, po):
        den0 = smallp.tile([1, 2, 512], F32, tag="den")
        nc.scalar.copy(out=den0[:], in_=po[64:65, :, :])
        ocp = ocpp.tile([65, 2, 512], F32, tag="ocp")
        nc.vector.tensor_copy(out=ocp[:], in_=po[0:65, :, :])
        rec = smallp.tile([1, 2, 512], F32, tag="rec")
        nc.vector.reciprocal_approx_fast(out=rec[:], in_=den0[:])
        return ocp, rec

    def norm_stage2(st):
        recbc = smallp.tile([64, 2, 512], F32, tag="recbc")
        nc.gpsimd.partition_broadcast(recbc[:], st["rec"][:])
        st["recbc"] = recbc

    def norm_stage3(st):
        qc, p, ocp, recbc = st["qc"], st["p"], st["ocp"], st["recbc"]
        # odd head first: normalize at base 0, DMA to partitions 64..127
        tmp = smallp.tile([64, 512], BF16, tag="otmp")
        nc.vector.tensor_tensor(
            out=tmp[:], in0=ocp[0:64, 1, :], in1=recbc[:, 1, :], op=ALU.mult,
        )
        nc.gpsimd.dma_start(
            s1T[64:128, p, qc * 512:(qc + 1) * 512], tmp[:]
        )
        nc.vector.tensor_tensor(
            out=s1T[0:64, p, qc * 512:(qc + 1) * 512],
            in0=ocp[0:64, 0, :], in1=recbc[:, 0, :], op=ALU.mult,
        )
        if st["after"] is not None:
            st["after"]()

    # Tail variant: nothing reuses po, so normalize straight out of PSUM
    # (no copy), the two heads' den->recip->broadcast chains are pipelined
    # with the odd head FIRST (it gates the partition-shift DMA), the
    # normalize is split into 128-col blocks so each outproj block's s1T
    # region completes (and its odd-head DMA, on the idle sync HWDGE queue,
    # flies) as early as possible, and small heater matmuls anchored on the
    # chain's own artifacts keep the PE HAM warm across the window.
    def emit_normalize_tail(qc, p, po):
        den1 = smallp.tile([1, 512], F32, tag="den")
        nc.scalar.copy(out=den1[:], in_=po[64:65, 1, :])
        rec1 = smallp.tile([1, 512], F32, tag="rec")
        nc.vector.reciprocal_approx_fast(out=rec1[:], in_=den1[:])
        den0 = smallp.tile([1, 512], F32, tag="den")
        nc.scalar.copy(out=den0[:], in_=po[64:65, 0, :])
        recbc1 = smallp.tile([64, 512], F32, tag="recbc")
        nc.gpsimd.partition_broadcast(recbc1[:], rec1[:])
        rec0 = smallp.tile([1, 512], F32, tag="rec")
        nc.vector.reciprocal_approx_fast(out=rec0[:], in_=den0[:])
        recbc0 = smallp.tile([64, 512], F32, tag="recbc")
        nc.gpsimd.partition_broadcast(recbc0[:], rec0[:])
        tmp = smallp.tile([64, 512], BF16, tag="otmp")
        for b in range(4):
            cs = slice(b * 128, (b + 1) * 128)
            col = slice(qc * 512 + b * 128, qc * 512 + (b + 1) * 128)
            nc.vector.tensor_tensor(
                out=tmp[:, cs], in0=po[0:64, 1, cs], in1=recbc1[:, cs],
                op=ALU.mult,
            )
            nc.sync.dma_start(s1T[64:128, p, col], tmp[:, cs])
            nc.vector.tensor_tensor(
                out=s1T[0:64, p, col],
                in0=po[0:64, 0, cs], in1=recbc0[:, cs], op=ALU.mult,
            )
            # heater pings anchored on the just-written blocks (the
            # scheduler cannot hoist them out of the normalize window)
            hp = ps_pp.tile([128, 512], F32, tag="pp")
            nc.tensor.matmul(
                hp[:, 0:128], lhsT=zdummy[0:64, :], rhs=tmp[:, cs],
                start=True, stop=True,
            )
            hp2 = ps_pp.tile([128, 512], F32, tag="pp")
            nc.tensor.matmul(
                hp2[:, 0:128], lhsT=zdummy[0:64, :], rhs=s1T[0:64, p, col],
                start=True, stop=True,
            )

    # deferred-normalize state: stage2 (broadcast) fires at the next pair's
    # kt=1, stage3 (multiplies + outproj enqueue) at its kt=2.
    pending_norm = []

    # ---- attention for one 512-query chunk ----
    def attention_chunk(qc, budget=0.6, last=False):
        n_k = 4 * qc + 4
        for p in range(2):
            ensure_g(("Q", qc, p))
            po = ps_po.tile([128, 2, 512], F32, tag="po")
            pend = []
            pt_cur = None
            for kt in range(n_k):
                ensure_g(("K", kt // 4, p))
                diag = kt >= 4 * qc
                q0 = 128 * (kt - 4 * qc) if diag else 0
                e = kt % 2
                if e == 0:
                    pt_cur = ptp.tile([128, 2, 2, 512], BF16, tag="pt")
                # scores for both heads of the pair: concurrent 64-row groups
                ps = ps_sc.tile([128, 2, 512], F32, tag="st")
                for h in range(2):
                    base = 64 * h
                    nc.tensor.matmul(
                        ps[:, h, q0:512],
                        lhsT=kT[p][base:base + 64, kt * 128:(kt + 1) * 128],
                        rhs=qT[p][base:base + 64, qc * 512 + q0:(qc + 1) * 512],
                        start=True, stop=True,
                    )
                # one exp for both heads
                nc.scalar.activation(
                    out=pt_cur[:, e, :, q0:512], in_=ps[:, :, q0:512],
                    func=AF.Exp, scale=SCALE,
                )
                if diag:
                    for h in range(2):
                        nc.vector.tensor_tensor(
                            out=pt_cur[:, e, h, q0:q0 + 128],
                            in0=pt_cur[:, e, h, q0:q0 + 128],
                            in1=mtri_sb[:],
                            op=ALU.mult,
                        )
                pend.append((kt, pt_cur, e, q0))
                # the previous pair's deferred normalize stages slot in here
                if kt == 1 and pending_norm:
                    norm_stage2(pending_norm[0])
                if kt == 2 and pending_norm:
                    norm_stage3(pending_norm.pop(0))
                # PV one step behind so exp(kt) overlaps scores(kt+1)
                if kt >= 1:
                    pkt, ptt, pe, pq0 = pend.pop(0)
                    ensure_g(("V", pkt // 4, pkt % 4))
                    for h in range(2):
                        hh = 2 * p + h
                        nc.tensor.matmul(
                            po[0:65, h, pq0:512],
                            lhsT=v_sb[:, pkt, HOFF[hh]:HOFF[hh] + 65],
                            rhs=ptt[:, pe, h, pq0:512],
                            start=(pkt == 0), stop=False,
                        )
                pop_fillers(budget)
            # final PV
            pkt, ptt, pe, pq0 = pend.pop(0)
            ensure_g(("V", pkt // 4, pkt % 4))
            for h in range(2):
                hh = 2 * p + h
                nc.tensor.matmul(
                    po[0:65, h, pq0:512],
                    lhsT=v_sb[:, pkt, HOFF[hh]:HOFF[hh] + 65],
                    rhs=ptt[:, pe, h, pq0:512],
                    start=(pkt == 0), stop=True,
                )
            if last and p == 1:
                # heater matmuls anchored on the last pt tile keep HAM warm
                # from the final PV into the normalize chain
                for _ in range(12):
                    hp = ps_pp.tile([128, 512], F32, tag="pp")
                    nc.tensor.matmul(
                        hp[:, 0:512], lhsT=zdummy[:], rhs=ptt[:, pe, 0, 0:512],
                        start=True, stop=True,
                    )
                emit_normalize_tail(qc, p, po)
            else:
                ocp, rec = norm_stage1(qc, p, po)
                pending_norm.append(
                    {"qc": qc, "p": p, "ocp": ocp, "rec": rec, "after": None}
                )

    # ---- the pipeline: ascending chunks; all projections ride along as
    # gated fillers, pulled just ahead of the attention steps that consume
    # them; output projections of earlier chunks fill later windows.  Each
    # chunk's outproj fillers are enqueued right after its second pair's
    # deferred normalize is emitted (emission order = dependency order). ----
    for qc in range(QC):
        add_proj_chunk(qc)
    attention_chunk(0, budget=0.7)
    pending_norm[-1]["after"] = lambda: add_outproj_chunk(0)
    attention_chunk(1, budget=0.7)
    pending_norm[-1]["after"] = lambda: add_outproj_chunk(1)
    attention_chunk(2, budget=0.8)
    pending_norm[-1]["after"] = lambda: add_outproj_chunk(2)
    nc.vector.tensor_copy(out=bo16[:], in_=bo_row[:])
    attention_chunk(3, budget=1.2, last=True)
    drain_fillers()
    add_outproj_chunk(3, use_sc=True, tail=True)
    drain_fillers()

    ctx.close()


_NC_CACHE = None


def _get_nc():
    global _NC_CACHE
    if _NC_CACHE is None:
        _NC_CACHE = _build()
    return _NC_CACHE


def _make_in_maps(x_q, x_k, x_v, Wq, bq, Wk, bk, Wv, bv, Wo, bo):
    f32 = np.float32
    bf16 = ml_dtypes.bfloat16
    mtri_np = np.triu(np.ones((128, 128), f32)).astype(bf16)

    # per-batch transposed inputs (shared by the 4 cores of each batch),
    # chunk-major SBUF-native: [128, QC*ET*512] with partition p holding,
    # per chunk, its ET 512-col rows packed contiguously.
    def x_layout(x):
        a = np.asarray(x, f32).reshape(QC, 512, ET, 128).transpose(3, 0, 2, 1)
        return np.ascontiguousarray(a.reshape(128, QC * ET * 512)).astype(bf16)

    xT = {}
    for b in range(B):
        xT[("q", b)] = x_layout(x_q[b])
        xT[("k", b)] = x_layout(x_k[b])
        xT[("v", b)] = x_layout(x_v[b])

    def sb_layout(w):
        """[D_EMB, n] -> [128, ET*n]: partition p holds rows {t*128+p} packed
        contiguously, so the DMA is one max-length line per partition."""
        n = w.shape[1]
        return np.ascontiguousarray(
            w.reshape(ET, 128, n).transpose(1, 0, 2).reshape(128, ET * n)
        )

    in_maps = []
    for core in range(NCORES):
        b, g = core // 4, core % 4
        sl = slice(g * DM_L, (g + 1) * DM_L)
        # augmented V weight/bias
        wv_aug = np.zeros((D_EMB, WV_AUG), f32)
        bv_aug = np.zeros((WV_AUG,), f32)
        for h in range(HG):
            gh = g * HG + h
            o = HOFF[h]
            wv_aug[:, o:o + 64] = Wv[:, gh * DH:(gh + 1) * DH]
            bv_aug[o:o + 64] = bv[gh * DH:(gh + 1) * DH]
            bv_aug[o + 64] = 1.0
        wo_sl = np.asarray(Wo[sl, :], f32)  # [256, 1024]
        wo_c = np.ascontiguousarray(
            wo_sl.reshape(2, 128, D_OUT).transpose(1, 0, 2).reshape(128, 2 * D_OUT)
        )
        in_maps.append({
            "xqT": xT[("q", b)],
            "xkT": xT[("k", b)],
            "xvT": xT[("v", b)],
            "wq": sb_layout(np.asarray(Wq[:, sl], f32)).astype(bf16),
            "wk": sb_layout(np.asarray(Wk[:, sl], f32)).astype(bf16),
            "wv": sb_layout(wv_aug).astype(bf16),
            "bq": np.ascontiguousarray(bq[sl], f32),
            "bk": np.ascontiguousarray(bk[sl], f32),
            "bv": bv_aug,
            "wo": wo_c.astype(bf16),
            # bias folded into the partials by exactly one core per group
            "bo": np.ascontiguousarray(bo, f32) if g == 0
                  else np.zeros((D_OUT,), f32),
            "mtri": mtri_np,
        })
    return in_maps


def run(inputs, trace=False, trace_kwargs=None):
    """Run on 8 NeuronCores. Returns (output [2,2048,1024] f32, BassKernelResults)."""
    inputs = {k: np.asarray(v) for k, v in inputs.items()}
    nc = _get_nc()
    in_maps = _make_in_maps(
        inputs["x_q"], inputs["x_k"], inputs["x_v"],
        inputs["Wq"], inputs["bq"], inputs["Wk"], inputs["bk"],
        inputs["Wv"], inputs["bv"], inputs["Wo"], inputs["bo"],
    )
    kwargs = {}
    if trace:
        kwargs["trace"] = True
        if trace_kwargs:
            kwargs.update(trace_kwargs)
    res = run_bass_kernel_spmd(nc, in_maps, core_ids=list(range(NCORES)), **kwargs)
    # unshard: each core holds a full-shape row-parallel partial for its
    # batch (4 head-groups per batch); summing them is the unshard step.
    out_full = np.zeros((B, S, D_OUT), np.float32)
    for core in range(NCORES):
        b = core // 4
        out_full[b] += np.asarray(res.results[core]["out"], np.float32)
    return out_full, res


def kernel(**inputs) -> np.ndarray:
    out, _ = run(inputs, trace=False)
    return out
